# revision 13
# baseline (speedup 1.0000x reference)
"""Multi-head attention Trainium2 kernel (8 NeuronCores, Bass/Tile).

Sharding: core c -> (batch b = c//2, head-group hg = c%2). Each core computes
attention for 8 of the 16 heads of one batch element plus its partial
out-projection; the host sums the two head-group partials per batch.

Per-core layouts (host pre-transposes inputs; contraction dims on partitions):
  xT  [E=1024, S=2048]      x[b].T
  wqT/wkT/wvT [1024, 512]   W[hg_rows].T
  woT [512, 1024]           Wo[:, hg_cols].T
  sel [8, 512]              0/1 selector for softmax-denominator replication

On-chip pipeline (all fp32):
  QT = wqT.T-tiles @ xT   [512, 2048] (head-major, transposed)
  KT likewise; V natural [2048, 512] with a ones-column appended per head
  scoresT[t,s] = KT_h.T-tile @ QT_h  (K=64, two heads row-packed per PE pass)
  expT = exp(scoresT/8) on ScalarE, batched [128, 2048] over 4 psum banks
  (outT | Z) = [V_h | 1].T @ expT    (M=65 matmul: row 64 = softmax sums)
  outT_norm = outT * replicate(1/Z)  (K=8 selector matmul + DVE mult)
  out = outT_norm.T-tiles @ woT      [2048, 1024] partial
"""

import os
import sys
import types

import numpy as np

B, S, E, H = 4, 2048, 1024, 16
DK = E // H  # 64
HG = H // 2  # heads per core = 8
DG = HG * DK  # 512 projected dims per core
NCORES = 8

TRACE = bool(os.environ.get("TRN_KERNEL_TRACE"))
LAST_EXEC_TIME_NS = None

_cache = {}


def _env_setup():
    import antenv

    if "antenv.axon_hooks" not in sys.modules:
        mod = types.ModuleType("antenv.axon_hooks")
        mod._hook = None
        mod.set_axon_ntff_profile_hook = lambda h: setattr(mod, "_hook", h)
        mod.get_axon_ntff_profile_hook = lambda: mod._hook
        sys.modules["antenv.axon_hooks"] = mod
        antenv.axon_hooks = mod
        try:
            from trn_agent_boot.trn_boot import _ntff_profile_via_ctypes

            mod.set_axon_ntff_profile_hook(
                _ntff_profile_via_ctypes("/opt/axon/libaxon_pjrt.so")
            )
        except Exception:
            pass

    import concourse.bass_utils as bass_utils

    bass_utils.upload_artifacts = lambda tmpdir: tmpdir

    import concourse.tile as tile
    from concourse import mybir
    from concourse.vector_clock import ScopedClock

    if getattr(tile.TileContext, "_wait_split_patched", False):
        return

    MAX_WAITS = 1  # walrus on this image rejects >1 sync wait per instruction

    def _drain_and_barrier_split(self, tick_clock, wait_clock):
        probe = self.nc.sync.drain()
        wait_clock.add_sem_waits(
            probe.ins, ScopedClock({None: tick_clock.global_clock})
        )
        waits = list(probe.ins.sync_info.on_wait)
        if len(waits) > MAX_WAITS:
            num2h = {h.num: h for h in self.sems.allocated().values()}
            probe.ins.sync_info.on_wait = []
            for w in waits:
                self.nc.sync.wait_ge(num2h[w.id], w.wait_value)
            self.nc.sync.drain()
        self.nc.all_engine_barrier()
        popped = self.nc._tile_sem_poison_stack.pop()
        assert popped is self._sem_poison
        self.nc.clear_and_free_semaphores(list(self.sems.allocated().values()))
        self.nc.all_engine_barrier()

    _orig_commit = tile.TileContext._commit_instruction
    _ctr = [0]

    def _commit_split_waits(self, inst, lazy_reg_writes=True):
        si = inst.sync_info
        if (
            si is not None
            and len(si.on_wait) > MAX_WAITS
            and inst.engine != mybir.EngineType.Unassigned
        ):
            waits = list(si.on_wait)
            keep, hoist = waits[:MAX_WAITS], waits[MAX_WAITS:]
            for i in range(0, len(hoist), MAX_WAITS):
                _ctr[0] += 1
                nop = mybir.InstNoOp(name=f"waitnop-{_ctr[0]}", ins=[], outs=[])
                nop.engine = inst.engine
                nop.sync_info = mybir.SyncInfo(
                    on_wait=hoist[i : i + MAX_WAITS], on_update=[]
                )
                self.nc.register_instruction(nop, overwrite=True)
                _orig_commit(self, nop, lazy_reg_writes=False)
            inst.sync_info = mybir.SyncInfo(on_wait=keep, on_update=list(si.on_update))
        return _orig_commit(self, inst, lazy_reg_writes=lazy_reg_writes)

    tile.TileContext._drain_and_barrier = _drain_and_barrier_split
    tile.TileContext._commit_instruction = _commit_split_waits
    tile.TileContext._wait_split_patched = True

    # use the full usable SBUF on trn2 (default constant is stale)
    import concourse.tile_utils as tile_utils

    tile_utils.max_sbuf_usage = 206 * 1024


def _build_nc():
    import concourse.bass as bass
    import concourse.tile as tile
    from concourse import mybir

    F32 = mybir.dt.float32
    PS = bass.MemorySpace.PSUM
    AF = mybir.ActivationFunctionType

    nc = bass.Bass()
    xT_d = nc.dram_tensor("xT", [E, S], F32, kind="ExternalInput")
    wqT_d = nc.dram_tensor("wqT", [E, DG], F32, kind="ExternalInput")
    wkT_d = nc.dram_tensor("wkT", [E, DG], F32, kind="ExternalInput")
    wvT_d = nc.dram_tensor("wvT", [E, DG], F32, kind="ExternalInput")
    woT_d = nc.dram_tensor("woT", [DG, E], F32, kind="ExternalInput")
    sel_d = nc.dram_tensor("sel", [HG, 512], F32, kind="ExternalInput")
    out_d = nc.dram_tensor("out", [S, E], F32, kind="ExternalOutput")

    NE = E // 128  # 8 e-tiles
    NT = S // 128  # 16 t/s-tiles
    NNC = S // 512  # 4 s-chunks
    NM = DG // 128  # 4 head-pair tiles

    with tile.TileContext(nc) as tc:
        with (
            tc.tile_pool(name="persist", bufs=1) as pp,
            tc.tile_pool(name="oppsum", bufs=2, space=PS) as op_ps,
            tc.tile_pool(name="scpsum", bufs=1, space=PS) as sc_ps,
            tc.tile_pool(name="avpsum", bufs=2, space=PS) as av_ps,
        ):
            sel_sb = pp.tile([HG, 512], F32, tag="sel")
            nc.sync.dma_start(sel_sb[:], sel_d[:])

            QT = pp.tile([128, NM * S], F32, tag="QT")  # [128, 8192]
            KT = pp.tile([128, NM * S], F32, tag="KT")
            Vsb = pp.tile([128, NT * (DG + HG)], F32, tag="V")  # [128, 16*520]

            # ones columns for the fused softmax-denominator trick
            nc.gpsimd.memset(Vsb[:], 1.0)

            # ---- projections ----
            with tc.tile_pool(name="xtp", bufs=1) as xtp:
                xT = xtp.tile([128, NE * S], F32, tag="xT")  # [128, 16384]
                for j in range(NE):
                    nc.sync.dma_start(
                        xT[:, j * S : (j + 1) * S], xT_d[j * 128 : (j + 1) * 128, :]
                    )

                for name, w_d, dst in (("q", wqT_d, QT), ("k", wkT_d, KT)):
                    with tc.tile_pool(name=f"w{name}", bufs=1) as wp:
                        wT = wp.tile([128, NE * DG], F32, tag=f"w{name}T")
                        for j in range(NE):
                            nc.sync.dma_start(
                                wT[:, j * DG : (j + 1) * DG],
                                w_d[j * 128 : (j + 1) * 128, :],
                            )
                        for m in range(NM):
                            for n in range(NNC):
                                acc = op_ps.tile([128, 512], F32, tag="acc")
                                for j in range(NE):
                                    nc.tensor.matmul(
                                        acc[:],
                                        wT[:, j * DG + m * 128 : j * DG + (m + 1) * 128],
                                        xT[:, j * S + n * 512 : j * S + (n + 1) * 512],
                                        start=(j == 0),
                                        stop=(j == NE - 1),
                                    )
                                nc.vector.tensor_copy(
                                    dst[:, m * S + n * 512 : m * S + (n + 1) * 512],
                                    acc[:],
                                )

                with tc.tile_pool(name="wv", bufs=1) as wp:
                    wvT = wp.tile([128, NE * DG], F32, tag="wvT")
                    for j in range(NE):
                        nc.sync.dma_start(
                            wvT[:, j * DG : (j + 1) * DG],
                            wvT_d[j * 128 : (j + 1) * 128, :],
                        )
                    for i in range(NT):
                        acc = op_ps.tile([128, 512], F32, tag="acc")
                        for j in range(NE):
                            nc.tensor.matmul(
                                acc[:],
                                xT[:, j * S + i * 128 : j * S + (i + 1) * 128],
                                wvT[:, j * DG : (j + 1) * DG],
                                start=(j == 0),
                                stop=(j == NE - 1),
                            )
                        # scatter 8 heads' 64 cols into 65-col slots (col 64 = ones)
                        dst = Vsb[
                            :, i * (DG + HG) : (i + 1) * (DG + HG)
                        ].rearrange("p (h c) -> p h c", c=DK + 1)[:, :, 0:DK]
                        src = acc[:].rearrange("p (h c) -> p h c", c=DK)
                        nc.vector.tensor_copy(dst, src)

            # attention-phase tensors/pools allocate after the xT pool frees
            import contextlib

            attn_stack = contextlib.ExitStack()
            pp2 = attn_stack.enter_context(tc.tile_pool(name="persist2", bufs=1))
            onorm = pp2.tile([128, NM * S], F32, tag="onorm")
            zbuf = pp2.tile([HG, S], F32, tag="zbuf")
            zrec = pp2.tile([HG, S], F32, tag="zrec")
            # engine ops need 32-aligned partition starts: stage Z rows at
            # partitions 0/64, then DMA-gather into zbuf's 8 partitions
            zstage = pp2.tile([128, NM * 512], F32, tag="zstage")
            woT = pp2.tile([128, NM * E], F32, tag="woT")  # [128, 4096]
            for k in range(NM):
                nc.sync.dma_start(
                    woT[:, k * E : (k + 1) * E], woT_d[k * 128 : (k + 1) * 128, :]
                )
            stg = attn_stack.enter_context(tc.tile_pool(name="stage", bufs=5))
            expp = attn_stack.enter_context(tc.tile_pool(name="expp", bufs=2))
            outp = attn_stack.enter_context(tc.tile_pool(name="outp", bufs=2))

            # ---- attention + out-projection, s-chunk major ----
            for n in range(NNC):
                uos = []
                for hp in range(NM):
                    av_h = av_ps.tile([65, 512], F32, tag="av")
                    av_h2 = av_ps.tile([65, 512], F32, tag="av")
                    for tg in range(NT // 2):
                        sc = sc_ps.tile([128, 2048], F32, tag="sc")
                        for dt in range(2):
                            t = 2 * tg + dt
                            nc.tensor.matmul(
                                sc[:, dt * 1024 : dt * 1024 + 512],
                                KT[0:64, hp * S + t * 128 : hp * S + (t + 1) * 128],
                                QT[0:64, hp * S + n * 512 : hp * S + (n + 1) * 512],
                            )
                            nc.tensor.matmul(
                                sc[:, dt * 1024 + 512 : dt * 1024 + 1024],
                                KT[64:128, hp * S + t * 128 : hp * S + (t + 1) * 128],
                                QT[64:128, hp * S + n * 512 : hp * S + (n + 1) * 512],
                            )
                        ex = expp.tile([128, 2048], F32, tag="ex")
                        nc.scalar.activation(ex[:], sc[:], AF.Exp, scale=0.125)
                        for dt in range(2):
                            t = 2 * tg + dt
                            voff = t * (DG + HG)
                            nc.tensor.matmul(
                                av_h[:],
                                Vsb[:, voff + (2 * hp) * 65 : voff + (2 * hp) * 65 + 65],
                                ex[:, dt * 1024 : dt * 1024 + 512],
                                start=(t == 0),
                                stop=(t == S // 128 - 1),
                            )
                            nc.tensor.matmul(
                                av_h2[:],
                                Vsb[
                                    :,
                                    voff + (2 * hp + 1) * 65 : voff + (2 * hp + 1) * 65 + 65,
                                ],
                                ex[:, dt * 1024 + 512 : dt * 1024 + 1024],
                                start=(t == 0),
                                stop=(t == S // 128 - 1),
                            )
                    uo = stg.tile([128, 512], F32, tag="uo")
                    nc.vector.tensor_copy(uo[0:64, :], av_h[0:64, :])
                    nc.vector.tensor_copy(uo[64:128, :], av_h2[0:64, :])
                    nc.vector.tensor_copy(
                        zstage[0:1, hp * 512 : (hp + 1) * 512], av_h[64:65, :]
                    )
                    nc.vector.tensor_copy(
                        zstage[64:65, hp * 512 : (hp + 1) * 512], av_h2[64:65, :]
                    )
                    uos.append(uo)

                # zbuf rows 0-3 = even heads (h of pair hp), rows 4-7 = odd heads
                for hp in range(NM):
                    nc.sync.dma_start(
                        zbuf[hp : hp + 1, n * 512 : (n + 1) * 512],
                        zstage[0:1, hp * 512 : (hp + 1) * 512],
                    )
                    nc.sync.dma_start(
                        zbuf[4 + hp : 5 + hp, n * 512 : (n + 1) * 512],
                        zstage[64:65, hp * 512 : (hp + 1) * 512],
                    )
                nc.vector.reciprocal(
                    zrec[:, n * 512 : (n + 1) * 512], zbuf[:, n * 512 : (n + 1) * 512]
                )
                for k in range(NM):
                    rep = op_ps.tile([128, 512], F32, tag="acc")
                    nc.tensor.matmul(
                        rep[:],
                        sel_sb[:, k * 128 : (k + 1) * 128],
                        zrec[:, n * 512 : (n + 1) * 512],
                    )
                    nc.vector.tensor_tensor(
                        onorm[:, k * S + n * 512 : k * S + (n + 1) * 512],
                        uos[k][:],
                        rep[:],
                        mybir.AluOpType.mult,
                    )

                for i in range(4 * n, 4 * n + 4):
                    ps0 = op_ps.tile([128, 512], F32, tag="acc")
                    ps1 = op_ps.tile([128, 512], F32, tag="acc")
                    for k in range(NM):
                        nc.tensor.matmul(
                            ps0[:],
                            onorm[:, k * S + i * 128 : k * S + (i + 1) * 128],
                            woT[:, k * E : k * E + 512],
                            start=(k == 0),
                            stop=(k == NM - 1),
                        )
                        nc.tensor.matmul(
                            ps1[:],
                            onorm[:, k * S + i * 128 : k * S + (i + 1) * 128],
                            woT[:, k * E + 512 : k * E + 1024],
                            start=(k == 0),
                            stop=(k == NM - 1),
                        )
                    osb = outp.tile([128, E], F32, tag="osb")
                    nc.vector.tensor_copy(osb[:, 0:512], ps0[:])
                    nc.vector.tensor_copy(osb[:, 512:1024], ps1[:])
                    nc.sync.dma_start(out_d[i * 128 : (i + 1) * 128, :], osb[:])

            attn_stack.close()

    return nc


def _make_sel():
    # zbuf row for head (2k + p//64): even heads -> row k, odd heads -> row 4+k
    sel = np.zeros((HG, 512), dtype=np.float32)
    for k in range(4):
        for p in range(128):
            r = k if p < 64 else 4 + k
            sel[r, k * 128 + p] = 1.0
    return sel


def kernel(x, Wq, Wk, Wv, Wo):
    global LAST_EXEC_TIME_NS
    _env_setup()
    from concourse.bass_utils import run_bass_kernel_spmd

    x = np.asarray(x, dtype=np.float32)
    Wq = np.asarray(Wq, dtype=np.float32)
    Wk = np.asarray(Wk, dtype=np.float32)
    Wv = np.asarray(Wv, dtype=np.float32)
    Wo = np.asarray(Wo, dtype=np.float32)

    if "nc" not in _cache:
        _cache["nc"] = _build_nc()
    nc = _cache["nc"]

    sel = _make_sel()
    in_maps = []
    for c in range(NCORES):
        b, hg = c // 2, c % 2
        r = slice(hg * DG, (hg + 1) * DG)
        in_maps.append(
            {
                "xT": np.ascontiguousarray(x[b].T),
                "wqT": np.ascontiguousarray(Wq[r, :].T),
                "wkT": np.ascontiguousarray(Wk[r, :].T),
                "wvT": np.ascontiguousarray(Wv[r, :].T),
                "woT": np.ascontiguousarray(Wo[:, r].T),
                "sel": sel,
            }
        )

    res = run_bass_kernel_spmd(
        nc, in_maps, core_ids=list(range(NCORES)), trace=TRACE
    )
    if TRACE:
        LAST_EXEC_TIME_NS = res.exec_time_ns

    out = np.empty((B, S, E), dtype=np.float32)
    for b in range(B):
        out[b] = res.results[2 * b]["out"] + res.results[2 * b + 1]["out"]
    return out


# revision 17
# speedup vs baseline: 2.1442x; 2.1442x over previous
"""Multi-head attention Trainium2 kernel (8 NeuronCores, Bass/Tile).

Sharding: core c -> (batch b = c//2, head-group hg = c%2). Each core computes
attention for 8 of the 16 heads of one batch element plus its partial
out-projection; the host sums the two head-group partials per batch.

Per-core layouts (host pre-transposes inputs; contraction dims on partitions):
  xT  [E=1024, S=2048]      x[b].T
  wqT/wkT/wvT [1024, 512]   W[hg_rows].T
  woT [512, 1024]           Wo[:, hg_cols].T
  sel [8, 512]              0/1 selector for softmax-denominator replication

On-chip pipeline (all fp32):
  QT = wqT.T-tiles @ xT   [512, 2048] (head-major, transposed)
  KT likewise; V natural [2048, 512] with a ones-column appended per head
  scoresT[t,s] = KT_h.T-tile @ QT_h  (K=64, two heads row-packed per PE pass)
  expT = exp(scoresT/8) on ScalarE, batched [128, 2048] over 4 psum banks
  (outT | Z) = [V_h | 1].T @ expT    (M=65 matmul: row 64 = softmax sums)
  outT_norm = outT * replicate(1/Z)  (K=8 selector matmul + DVE mult)
  out = outT_norm.T-tiles @ woT      [2048, 1024] partial
"""

import os
import sys
import types

import numpy as np

B, S, E, H = 4, 2048, 1024, 16
DK = E // H  # 64
HG = H // 2  # heads per core = 8
DG = HG * DK  # 512 projected dims per core
NCORES = 8

TRACE = bool(os.environ.get("TRN_KERNEL_TRACE"))
# matmul-operand dtype: bf16 single-pass PE (fp32 PSUM accumulation) vs
# fp32 operands (PE double-pumps each matmul -> ~2x slower)
MM_DTYPE = os.environ.get("TRN_MM_DTYPE", "bf16")
LAST_EXEC_TIME_NS = None

_cache = {}


def _env_setup():
    import antenv

    if "antenv.axon_hooks" not in sys.modules:
        mod = types.ModuleType("antenv.axon_hooks")
        mod._hook = None
        mod.set_axon_ntff_profile_hook = lambda h: setattr(mod, "_hook", h)
        mod.get_axon_ntff_profile_hook = lambda: mod._hook
        sys.modules["antenv.axon_hooks"] = mod
        antenv.axon_hooks = mod
        try:
            from trn_agent_boot.trn_boot import _ntff_profile_via_ctypes

            mod.set_axon_ntff_profile_hook(
                _ntff_profile_via_ctypes("/opt/axon/libaxon_pjrt.so")
            )
        except Exception:
            pass

    import concourse.bass_utils as bass_utils

    bass_utils.upload_artifacts = lambda tmpdir: tmpdir

    import concourse.tile as tile
    from concourse import mybir
    from concourse.vector_clock import ScopedClock

    if getattr(tile.TileContext, "_wait_split_patched", False):
        return

    MAX_WAITS = 1  # walrus on this image rejects >1 sync wait per instruction

    def _drain_and_barrier_split(self, tick_clock, wait_clock):
        probe = self.nc.sync.drain()
        wait_clock.add_sem_waits(
            probe.ins, ScopedClock({None: tick_clock.global_clock})
        )
        waits = list(probe.ins.sync_info.on_wait)
        if len(waits) > MAX_WAITS:
            num2h = {h.num: h for h in self.sems.allocated().values()}
            probe.ins.sync_info.on_wait = []
            for w in waits:
                self.nc.sync.wait_ge(num2h[w.id], w.wait_value)
            self.nc.sync.drain()
        self.nc.all_engine_barrier()
        popped = self.nc._tile_sem_poison_stack.pop()
        assert popped is self._sem_poison
        self.nc.clear_and_free_semaphores(list(self.sems.allocated().values()))
        self.nc.all_engine_barrier()

    _orig_commit = tile.TileContext._commit_instruction
    _ctr = [0]

    def _commit_split_waits(self, inst, lazy_reg_writes=True):
        si = inst.sync_info
        if (
            si is not None
            and len(si.on_wait) > MAX_WAITS
            and inst.engine != mybir.EngineType.Unassigned
        ):
            waits = list(si.on_wait)
            keep, hoist = waits[:MAX_WAITS], waits[MAX_WAITS:]
            for i in range(0, len(hoist), MAX_WAITS):
                _ctr[0] += 1
                nop = mybir.InstNoOp(name=f"waitnop-{_ctr[0]}", ins=[], outs=[])
                nop.engine = inst.engine
                nop.sync_info = mybir.SyncInfo(
                    on_wait=hoist[i : i + MAX_WAITS], on_update=[]
                )
                self.nc.register_instruction(nop, overwrite=True)
                _orig_commit(self, nop, lazy_reg_writes=False)
            inst.sync_info = mybir.SyncInfo(on_wait=keep, on_update=list(si.on_update))
        return _orig_commit(self, inst, lazy_reg_writes=lazy_reg_writes)

    tile.TileContext._drain_and_barrier = _drain_and_barrier_split
    tile.TileContext._commit_instruction = _commit_split_waits
    tile.TileContext._wait_split_patched = True

    # use the full usable SBUF on trn2 (default constant is stale)
    import concourse.tile_utils as tile_utils

    tile_utils.max_sbuf_usage = 206 * 1024


def _build_nc():
    import concourse.bass as bass
    import concourse.tile as tile
    from concourse import mybir

    F32 = mybir.dt.float32
    CDT = mybir.dt.bfloat16 if MM_DTYPE == "bf16" else mybir.dt.float32
    PS = bass.MemorySpace.PSUM
    AF = mybir.ActivationFunctionType

    nc = bass.Bass()
    xT_d = nc.dram_tensor("xT", [E, S], CDT, kind="ExternalInput")
    wqT_d = nc.dram_tensor("wqT", [E, DG], CDT, kind="ExternalInput")
    wkT_d = nc.dram_tensor("wkT", [E, DG], CDT, kind="ExternalInput")
    wvT_d = nc.dram_tensor("wvT", [E, DG], CDT, kind="ExternalInput")
    woT_d = nc.dram_tensor("woT", [DG, E], CDT, kind="ExternalInput")
    sel_d = nc.dram_tensor("sel", [HG, 512], F32, kind="ExternalInput")
    out_d = nc.dram_tensor("out", [S, E], F32, kind="ExternalOutput")

    NE = E // 128  # 8 e-tiles
    NT = S // 128  # 16 t/s-tiles
    NNC = S // 512  # 4 s-chunks
    NM = DG // 128  # 4 head-pair tiles

    with tile.TileContext(nc) as tc:
        with (
            tc.tile_pool(name="persist", bufs=1) as pp,
            tc.tile_pool(name="oppsum", bufs=2, space=PS) as op_ps,
            tc.tile_pool(name="scpsum", bufs=1, space=PS) as sc_ps,
            tc.tile_pool(name="avpsum", bufs=2, space=PS) as av_ps,
        ):
            sel_sb = pp.tile([HG, 512], F32, tag="sel")
            nc.sync.dma_start(sel_sb[:], sel_d[:])

            QT = pp.tile([128, NM * S], CDT, tag="QT")  # [128, 8192]
            KT = pp.tile([128, NM * S], CDT, tag="KT")
            Vsb = pp.tile([128, NT * (DG + HG)], CDT, tag="V")  # [128, 16*520]

            # ones columns for the fused softmax-denominator trick
            nc.gpsimd.memset(Vsb[:], 1.0)

            # ---- projections ----
            with tc.tile_pool(name="xtp", bufs=1) as xtp:
                xT = xtp.tile([128, NE * S], CDT, tag="xT")  # [128, 16384]
                for j in range(NE):
                    nc.sync.dma_start(
                        xT[:, j * S : (j + 1) * S], xT_d[j * 128 : (j + 1) * 128, :]
                    )

                for name, w_d, dst in (("q", wqT_d, QT), ("k", wkT_d, KT)):
                    with tc.tile_pool(name=f"w{name}", bufs=1) as wp:
                        wT = wp.tile([128, NE * DG], CDT, tag=f"w{name}T")
                        for j in range(NE):
                            nc.sync.dma_start(
                                wT[:, j * DG : (j + 1) * DG],
                                w_d[j * 128 : (j + 1) * 128, :],
                            )
                        for m in range(NM):
                            for n in range(NNC):
                                acc = op_ps.tile([128, 512], F32, tag="acc")
                                for j in range(NE):
                                    nc.tensor.matmul(
                                        acc[:],
                                        wT[:, j * DG + m * 128 : j * DG + (m + 1) * 128],
                                        xT[:, j * S + n * 512 : j * S + (n + 1) * 512],
                                        start=(j == 0),
                                        stop=(j == NE - 1),
                                    )
                                nc.vector.tensor_copy(
                                    dst[:, m * S + n * 512 : m * S + (n + 1) * 512],
                                    acc[:],
                                )

                with tc.tile_pool(name="wv", bufs=1) as wp:
                    wvT = wp.tile([128, NE * DG], CDT, tag="wvT")
                    for j in range(NE):
                        nc.sync.dma_start(
                            wvT[:, j * DG : (j + 1) * DG],
                            wvT_d[j * 128 : (j + 1) * 128, :],
                        )
                    for i in range(NT):
                        acc = op_ps.tile([128, 512], F32, tag="acc")
                        for j in range(NE):
                            nc.tensor.matmul(
                                acc[:],
                                xT[:, j * S + i * 128 : j * S + (i + 1) * 128],
                                wvT[:, j * DG : (j + 1) * DG],
                                start=(j == 0),
                                stop=(j == NE - 1),
                            )
                        # scatter 8 heads' 64 cols into 65-col slots (col 64 = ones)
                        dst = Vsb[
                            :, i * (DG + HG) : (i + 1) * (DG + HG)
                        ].rearrange("p (h c) -> p h c", c=DK + 1)[:, :, 0:DK]
                        src = acc[:].rearrange("p (h c) -> p h c", c=DK)
                        nc.vector.tensor_copy(dst, src)

            # attention-phase tensors/pools allocate after the xT pool frees
            import contextlib

            attn_stack = contextlib.ExitStack()
            pp2 = attn_stack.enter_context(tc.tile_pool(name="persist2", bufs=1))
            onorm = pp2.tile([128, NM * S], CDT, tag="onorm")
            zbuf = pp2.tile([HG, S], F32, tag="zbuf")
            zrec = pp2.tile([HG, S], F32, tag="zrec")
            # engine ops need 32-aligned partition starts: stage Z rows at
            # partitions 0/64, then DMA-gather into zbuf's 8 partitions
            zstage = pp2.tile([128, NM * 512], F32, tag="zstage")
            woT = pp2.tile([128, NM * E], CDT, tag="woT")  # [128, 4096]
            for k in range(NM):
                nc.sync.dma_start(
                    woT[:, k * E : (k + 1) * E], woT_d[k * 128 : (k + 1) * 128, :]
                )
            stg = attn_stack.enter_context(tc.tile_pool(name="stage", bufs=5))
            expp = attn_stack.enter_context(tc.tile_pool(name="expp", bufs=2))
            outp = attn_stack.enter_context(tc.tile_pool(name="outp", bufs=2))

            # ---- attention + out-projection, s-chunk major ----
            for n in range(NNC):
                uos = []
                for hp in range(NM):
                    av_h = av_ps.tile([65, 512], F32, tag="av")
                    av_h2 = av_ps.tile([65, 512], F32, tag="av")
                    for tg in range(NT // 2):
                        sc = sc_ps.tile([128, 2048], F32, tag="sc")
                        for dt in range(2):
                            t = 2 * tg + dt
                            nc.tensor.matmul(
                                sc[:, dt * 1024 : dt * 1024 + 512],
                                KT[0:64, hp * S + t * 128 : hp * S + (t + 1) * 128],
                                QT[0:64, hp * S + n * 512 : hp * S + (n + 1) * 512],
                            )
                            nc.tensor.matmul(
                                sc[:, dt * 1024 + 512 : dt * 1024 + 1024],
                                KT[64:128, hp * S + t * 128 : hp * S + (t + 1) * 128],
                                QT[64:128, hp * S + n * 512 : hp * S + (n + 1) * 512],
                            )
                        ex = expp.tile([128, 2048], CDT, tag="ex")
                        nc.scalar.activation(ex[:], sc[:], AF.Exp, scale=0.125)
                        for dt in range(2):
                            t = 2 * tg + dt
                            voff = t * (DG + HG)
                            nc.tensor.matmul(
                                av_h[:],
                                Vsb[:, voff + (2 * hp) * 65 : voff + (2 * hp) * 65 + 65],
                                ex[:, dt * 1024 : dt * 1024 + 512],
                                start=(t == 0),
                                stop=(t == S // 128 - 1),
                            )
                            nc.tensor.matmul(
                                av_h2[:],
                                Vsb[
                                    :,
                                    voff + (2 * hp + 1) * 65 : voff + (2 * hp + 1) * 65 + 65,
                                ],
                                ex[:, dt * 1024 + 512 : dt * 1024 + 1024],
                                start=(t == 0),
                                stop=(t == S // 128 - 1),
                            )
                    uo = stg.tile([128, 512], F32, tag="uo")
                    nc.vector.tensor_copy(uo[0:64, :], av_h[0:64, :])
                    nc.vector.tensor_copy(uo[64:128, :], av_h2[0:64, :])
                    nc.vector.tensor_copy(
                        zstage[0:1, hp * 512 : (hp + 1) * 512], av_h[64:65, :]
                    )
                    nc.vector.tensor_copy(
                        zstage[64:65, hp * 512 : (hp + 1) * 512], av_h2[64:65, :]
                    )
                    uos.append(uo)

                # zbuf rows 0-3 = even heads (h of pair hp), rows 4-7 = odd heads
                for hp in range(NM):
                    nc.sync.dma_start(
                        zbuf[hp : hp + 1, n * 512 : (n + 1) * 512],
                        zstage[0:1, hp * 512 : (hp + 1) * 512],
                    )
                    nc.sync.dma_start(
                        zbuf[4 + hp : 5 + hp, n * 512 : (n + 1) * 512],
                        zstage[64:65, hp * 512 : (hp + 1) * 512],
                    )
                nc.vector.reciprocal(
                    zrec[:, n * 512 : (n + 1) * 512], zbuf[:, n * 512 : (n + 1) * 512]
                )
                for k in range(NM):
                    rep = op_ps.tile([128, 512], F32, tag="acc")
                    nc.tensor.matmul(
                        rep[:],
                        sel_sb[:, k * 128 : (k + 1) * 128],
                        zrec[:, n * 512 : (n + 1) * 512],
                    )
                    nc.vector.tensor_tensor(
                        onorm[:, k * S + n * 512 : k * S + (n + 1) * 512],
                        uos[k][:],
                        rep[:],
                        mybir.AluOpType.mult,
                    )

                for i in range(4 * n, 4 * n + 4):
                    ps0 = op_ps.tile([128, 512], F32, tag="acc")
                    ps1 = op_ps.tile([128, 512], F32, tag="acc")
                    for k in range(NM):
                        nc.tensor.matmul(
                            ps0[:],
                            onorm[:, k * S + i * 128 : k * S + (i + 1) * 128],
                            woT[:, k * E : k * E + 512],
                            start=(k == 0),
                            stop=(k == NM - 1),
                        )
                        nc.tensor.matmul(
                            ps1[:],
                            onorm[:, k * S + i * 128 : k * S + (i + 1) * 128],
                            woT[:, k * E + 512 : k * E + 1024],
                            start=(k == 0),
                            stop=(k == NM - 1),
                        )
                    osb = outp.tile([128, E], F32, tag="osb")
                    nc.vector.tensor_copy(osb[:, 0:512], ps0[:])
                    nc.vector.tensor_copy(osb[:, 512:1024], ps1[:])
                    nc.sync.dma_start(out_d[i * 128 : (i + 1) * 128, :], osb[:])

            attn_stack.close()

    return nc


def _make_sel():
    # zbuf row for head (2k + p//64): even heads -> row k, odd heads -> row 4+k
    sel = np.zeros((HG, 512), dtype=np.float32)
    for k in range(4):
        for p in range(128):
            r = k if p < 64 else 4 + k
            sel[r, k * 128 + p] = 1.0
    return sel


def kernel(x, Wq, Wk, Wv, Wo):
    global LAST_EXEC_TIME_NS
    _env_setup()
    from concourse.bass_utils import run_bass_kernel_spmd

    x = np.asarray(x, dtype=np.float32)
    Wq = np.asarray(Wq, dtype=np.float32)
    Wk = np.asarray(Wk, dtype=np.float32)
    Wv = np.asarray(Wv, dtype=np.float32)
    Wo = np.asarray(Wo, dtype=np.float32)

    if "nc" not in _cache:
        _cache["nc"] = _build_nc()
    nc = _cache["nc"]

    if MM_DTYPE == "bf16":
        import ml_dtypes

        cdt = ml_dtypes.bfloat16
    else:
        cdt = np.float32

    sel = _make_sel()
    in_maps = []
    for c in range(NCORES):
        b, hg = c // 2, c % 2
        r = slice(hg * DG, (hg + 1) * DG)
        in_maps.append(
            {
                "xT": np.ascontiguousarray(x[b].T).astype(cdt),
                "wqT": np.ascontiguousarray(Wq[r, :].T).astype(cdt),
                "wkT": np.ascontiguousarray(Wk[r, :].T).astype(cdt),
                "wvT": np.ascontiguousarray(Wv[r, :].T).astype(cdt),
                "woT": np.ascontiguousarray(Wo[:, r].T).astype(cdt),
                "sel": sel,
            }
        )

    res = run_bass_kernel_spmd(
        nc, in_maps, core_ids=list(range(NCORES)), trace=TRACE
    )
    if TRACE:
        LAST_EXEC_TIME_NS = res.exec_time_ns

    out = np.empty((B, S, E), dtype=np.float32)
    for b in range(B):
        out[b] = res.results[2 * b]["out"] + res.results[2 * b + 1]["out"]
    return out


# revision 18
# speedup vs baseline: 2.6123x; 1.2183x over previous
"""Multi-head attention Trainium2 kernel (8 NeuronCores, Bass/Tile).

Sharding: core c -> (batch b = c//2, head-group hg = c%2). Each core computes
attention for 8 of the 16 heads of one batch element plus its partial
out-projection; the host sums the two head-group partials per batch.

Per-core layouts (host pre-transposes inputs; contraction dims on partitions):
  xT  [E=1024, S=2048]      x[b].T
  wqT/wkT/wvT [1024, 512]   W[hg_rows].T
  woT [512, 1024]           Wo[:, hg_cols].T
  sel [8, 512]              0/1 selector for softmax-denominator replication

On-chip pipeline (all fp32):
  QT = wqT.T-tiles @ xT   [512, 2048] (head-major, transposed)
  KT likewise; V natural [2048, 512] with a ones-column appended per head
  scoresT[t,s] = KT_h.T-tile @ QT_h  (K=64, two heads row-packed per PE pass)
  expT = exp(scoresT/8) on ScalarE, batched [128, 2048] over 4 psum banks
  (outT | Z) = [V_h | 1].T @ expT    (M=65 matmul: row 64 = softmax sums)
  outT_norm = outT * replicate(1/Z)  (K=8 selector matmul + DVE mult)
  out = outT_norm.T-tiles @ woT      [2048, 1024] partial
"""

import os
import sys
import types

import numpy as np

B, S, E, H = 4, 2048, 1024, 16
DK = E // H  # 64
HG = H // 2  # heads per core = 8
DG = HG * DK  # 512 projected dims per core
NCORES = 8

TRACE = bool(os.environ.get("TRN_KERNEL_TRACE"))
# matmul-operand dtype: bf16 single-pass PE (fp32 PSUM accumulation) vs
# fp32 operands (PE double-pumps each matmul -> ~2x slower)
MM_DTYPE = os.environ.get("TRN_MM_DTYPE", "bf16")
LAST_EXEC_TIME_NS = None

_cache = {}


def _env_setup():
    import antenv

    if "antenv.axon_hooks" not in sys.modules:
        mod = types.ModuleType("antenv.axon_hooks")
        mod._hook = None
        mod.set_axon_ntff_profile_hook = lambda h: setattr(mod, "_hook", h)
        mod.get_axon_ntff_profile_hook = lambda: mod._hook
        sys.modules["antenv.axon_hooks"] = mod
        antenv.axon_hooks = mod
        try:
            from trn_agent_boot.trn_boot import _ntff_profile_via_ctypes

            mod.set_axon_ntff_profile_hook(
                _ntff_profile_via_ctypes("/opt/axon/libaxon_pjrt.so")
            )
        except Exception:
            pass

    import concourse.bass_utils as bass_utils

    bass_utils.upload_artifacts = lambda tmpdir: tmpdir

    import concourse.tile as tile
    from concourse import mybir
    from concourse.vector_clock import ScopedClock

    if getattr(tile.TileContext, "_wait_split_patched", False):
        return

    MAX_WAITS = 1  # walrus on this image rejects >1 sync wait per instruction

    def _drain_and_barrier_split(self, tick_clock, wait_clock):
        probe = self.nc.sync.drain()
        wait_clock.add_sem_waits(
            probe.ins, ScopedClock({None: tick_clock.global_clock})
        )
        waits = list(probe.ins.sync_info.on_wait)
        if len(waits) > MAX_WAITS:
            num2h = {h.num: h for h in self.sems.allocated().values()}
            probe.ins.sync_info.on_wait = []
            for w in waits:
                self.nc.sync.wait_ge(num2h[w.id], w.wait_value)
            self.nc.sync.drain()
        self.nc.all_engine_barrier()
        popped = self.nc._tile_sem_poison_stack.pop()
        assert popped is self._sem_poison
        self.nc.clear_and_free_semaphores(list(self.sems.allocated().values()))
        self.nc.all_engine_barrier()

    _orig_commit = tile.TileContext._commit_instruction
    _ctr = [0]

    def _commit_split_waits(self, inst, lazy_reg_writes=True):
        si = inst.sync_info
        if (
            si is not None
            and len(si.on_wait) > MAX_WAITS
            and inst.engine != mybir.EngineType.Unassigned
        ):
            waits = list(si.on_wait)
            keep, hoist = waits[:MAX_WAITS], waits[MAX_WAITS:]
            for i in range(0, len(hoist), MAX_WAITS):
                _ctr[0] += 1
                nop = mybir.InstNoOp(name=f"waitnop-{_ctr[0]}", ins=[], outs=[])
                nop.engine = inst.engine
                nop.sync_info = mybir.SyncInfo(
                    on_wait=hoist[i : i + MAX_WAITS], on_update=[]
                )
                self.nc.register_instruction(nop, overwrite=True)
                _orig_commit(self, nop, lazy_reg_writes=False)
            inst.sync_info = mybir.SyncInfo(on_wait=keep, on_update=list(si.on_update))
        return _orig_commit(self, inst, lazy_reg_writes=lazy_reg_writes)

    tile.TileContext._drain_and_barrier = _drain_and_barrier_split
    tile.TileContext._commit_instruction = _commit_split_waits
    tile.TileContext._wait_split_patched = True

    # use the full usable SBUF on trn2 (default constant is stale)
    import concourse.tile_utils as tile_utils

    tile_utils.max_sbuf_usage = 206 * 1024


def _build_nc():
    import concourse.bass as bass
    import concourse.tile as tile
    from concourse import mybir

    F32 = mybir.dt.float32
    CDT = mybir.dt.bfloat16 if MM_DTYPE == "bf16" else mybir.dt.float32
    PS = bass.MemorySpace.PSUM
    AF = mybir.ActivationFunctionType

    nc = bass.Bass()
    xT_d = nc.dram_tensor("xT", [E, S], CDT, kind="ExternalInput")
    wqT_d = nc.dram_tensor("wqT", [E, DG], CDT, kind="ExternalInput")
    wkT_d = nc.dram_tensor("wkT", [E, DG], CDT, kind="ExternalInput")
    wvT_d = nc.dram_tensor("wvT", [E, DG], CDT, kind="ExternalInput")
    woT_d = nc.dram_tensor("woT", [DG, E], CDT, kind="ExternalInput")
    sel_d = nc.dram_tensor("sel", [HG, 512], F32, kind="ExternalInput")
    out_d = nc.dram_tensor("out", [S, E], F32, kind="ExternalOutput")

    NE = E // 128  # 8 e-tiles
    NT = S // 128  # 16 t/s-tiles
    NNC = S // 512  # 4 s-chunks
    NM = DG // 128  # 4 head-pair tiles

    with tile.TileContext(nc) as tc:
        with (
            tc.tile_pool(name="persist", bufs=1) as pp,
            tc.tile_pool(name="oppsum", bufs=2, space=PS) as op_ps,
            tc.tile_pool(name="scpsum", bufs=2, space=PS) as sc_ps,
            tc.tile_pool(name="avpsum", bufs=1, space=PS) as av_ps,
            tc.tile_pool(name="zpsum", bufs=1, space=PS) as z_ps,
        ):
            sel_sb = pp.tile([HG, 512], F32, tag="sel")
            nc.sync.dma_start(sel_sb[:], sel_d[:])

            QT = pp.tile([128, NM * S], CDT, tag="QT")  # [128, 8192]
            KT = pp.tile([128, NM * S], CDT, tag="KT")
            Vsb = pp.tile([128, NT * DG], CDT, tag="V")  # [128, 8192]
            ones = pp.tile([128, 1], CDT, tag="ones")
            nc.gpsimd.memset(ones[:], 1.0)

            # ---- projections ----
            with tc.tile_pool(name="xtp", bufs=1) as xtp:
                xT = xtp.tile([128, NE * S], CDT, tag="xT")  # [128, 16384]
                for j in range(NE):
                    nc.sync.dma_start(
                        xT[:, j * S : (j + 1) * S], xT_d[j * 128 : (j + 1) * 128, :]
                    )

                for name, w_d, dst in (("q", wqT_d, QT), ("k", wkT_d, KT)):
                    with tc.tile_pool(name=f"w{name}", bufs=1) as wp:
                        wT = wp.tile([128, NE * DG], CDT, tag=f"w{name}T")
                        for j in range(NE):
                            nc.sync.dma_start(
                                wT[:, j * DG : (j + 1) * DG],
                                w_d[j * 128 : (j + 1) * 128, :],
                            )
                        for m in range(NM):
                            for n in range(NNC):
                                acc = op_ps.tile([128, 512], F32, tag="acc")
                                for j in range(NE):
                                    nc.tensor.matmul(
                                        acc[:],
                                        wT[:, j * DG + m * 128 : j * DG + (m + 1) * 128],
                                        xT[:, j * S + n * 512 : j * S + (n + 1) * 512],
                                        start=(j == 0),
                                        stop=(j == NE - 1),
                                    )
                                nc.vector.tensor_copy(
                                    dst[:, m * S + n * 512 : m * S + (n + 1) * 512],
                                    acc[:],
                                )

                with tc.tile_pool(name="wv", bufs=1) as wp:
                    wvT = wp.tile([128, NE * DG], CDT, tag="wvT")
                    for j in range(NE):
                        nc.sync.dma_start(
                            wvT[:, j * DG : (j + 1) * DG],
                            wvT_d[j * 128 : (j + 1) * 128, :],
                        )
                    for i in range(NT):
                        acc = op_ps.tile([128, 512], F32, tag="acc")
                        for j in range(NE):
                            nc.tensor.matmul(
                                acc[:],
                                xT[:, j * S + i * 128 : j * S + (i + 1) * 128],
                                wvT[:, j * DG : (j + 1) * DG],
                                start=(j == 0),
                                stop=(j == NE - 1),
                            )
                        nc.vector.tensor_copy(
                            Vsb[:, i * DG : (i + 1) * DG], acc[:]
                        )

            # attention-phase tensors/pools allocate after the xT pool frees
            import contextlib

            attn_stack = contextlib.ExitStack()
            pp2 = attn_stack.enter_context(tc.tile_pool(name="persist2", bufs=1))
            onorm = pp2.tile([128, NM * S], CDT, tag="onorm")
            zbuf = pp2.tile([HG, S], F32, tag="zbuf")
            zrec = pp2.tile([HG, S], F32, tag="zrec")
            # engine ops need 32-aligned partition starts: stage Z rows at
            # partitions 0/64, then DMA-gather into zbuf's 8 partitions
            zstage = pp2.tile([128, NM * 512], F32, tag="zstage")
            woT = pp2.tile([128, NM * E], CDT, tag="woT")  # [128, 4096]
            for k in range(NM):
                nc.sync.dma_start(
                    woT[:, k * E : (k + 1) * E], woT_d[k * 128 : (k + 1) * 128, :]
                )
            stg = attn_stack.enter_context(tc.tile_pool(name="stage", bufs=5))
            expp = attn_stack.enter_context(tc.tile_pool(name="expp", bufs=2))
            outp = attn_stack.enter_context(tc.tile_pool(name="outp", bufs=2))

            # ---- attention + out-projection, s-chunk major ----
            for n in range(NNC):
                uos = []
                for hp in range(NM):
                    av = av_ps.tile([128, 512], F32, tag="av")
                    zz = z_ps.tile([33, 512], F32, tag="zz")
                    for t in range(NT):
                        sc = sc_ps.tile([128, 1024], F32, tag="sc")
                        nc.tensor.matmul(
                            sc[:, 0:512],
                            KT[0:64, hp * S + t * 128 : hp * S + (t + 1) * 128],
                            QT[0:64, hp * S + n * 512 : hp * S + (n + 1) * 512],
                        )
                        nc.tensor.matmul(
                            sc[:, 512:1024],
                            KT[64:128, hp * S + t * 128 : hp * S + (t + 1) * 128],
                            QT[64:128, hp * S + n * 512 : hp * S + (n + 1) * 512],
                        )
                        ex = expp.tile([128, 1024], CDT, tag="ex")
                        nc.scalar.activation(ex[:], sc[:], AF.Exp, scale=0.125)
                        voff = t * DG
                        # col-packed AV pair (concurrent: col groups 0-1 / 2-3)
                        nc.tensor.matmul(
                            av[0:64, :],
                            Vsb[:, voff + (2 * hp) * DK : voff + (2 * hp) * DK + DK],
                            ex[:, 0:512],
                            start=(t == 0),
                            stop=(t == NT - 1),
                            tile_position=(0, 0),
                            skip_group_check=True,
                        )
                        nc.tensor.matmul(
                            av[64:128, :],
                            Vsb[
                                :,
                                voff + (2 * hp + 1) * DK : voff + (2 * hp + 1) * DK + DK,
                            ],
                            ex[:, 512:1024],
                            start=(t == 0),
                            stop=(t == NT - 1),
                            tile_position=(0, 64),
                            skip_group_check=True,
                        )
                        # softmax denominators via ones-column (concurrent pair)
                        nc.tensor.matmul(
                            zz[0:1, :],
                            ones[:, 0:1],
                            ex[:, 0:512],
                            start=(t == 0),
                            stop=(t == NT - 1),
                            tile_position=(0, 0),
                            skip_group_check=True,
                        )
                        nc.tensor.matmul(
                            zz[32:33, :],
                            ones[:, 0:1],
                            ex[:, 512:1024],
                            start=(t == 0),
                            stop=(t == NT - 1),
                            tile_position=(0, 32),
                            skip_group_check=True,
                        )
                    uo = stg.tile([128, 512], F32, tag="uo")
                    nc.vector.tensor_copy(uo[:], av[:])
                    nc.vector.tensor_copy(
                        zstage[0:1, hp * 512 : (hp + 1) * 512], zz[0:1, :]
                    )
                    nc.vector.tensor_copy(
                        zstage[32:33, hp * 512 : (hp + 1) * 512], zz[32:33, :]
                    )
                    uos.append(uo)

                # zbuf rows 0-3 = even heads (h of pair hp), rows 4-7 = odd heads
                for hp in range(NM):
                    nc.sync.dma_start(
                        zbuf[hp : hp + 1, n * 512 : (n + 1) * 512],
                        zstage[0:1, hp * 512 : (hp + 1) * 512],
                    )
                    nc.sync.dma_start(
                        zbuf[4 + hp : 5 + hp, n * 512 : (n + 1) * 512],
                        zstage[32:33, hp * 512 : (hp + 1) * 512],
                    )
                nc.vector.reciprocal(
                    zrec[:, n * 512 : (n + 1) * 512], zbuf[:, n * 512 : (n + 1) * 512]
                )
                for k in range(NM):
                    rep = op_ps.tile([128, 512], F32, tag="acc")
                    nc.tensor.matmul(
                        rep[:],
                        sel_sb[:, k * 128 : (k + 1) * 128],
                        zrec[:, n * 512 : (n + 1) * 512],
                    )
                    nc.vector.tensor_tensor(
                        onorm[:, k * S + n * 512 : k * S + (n + 1) * 512],
                        uos[k][:],
                        rep[:],
                        mybir.AluOpType.mult,
                    )

                for i in range(4 * n, 4 * n + 4):
                    ps0 = op_ps.tile([128, 512], F32, tag="acc")
                    ps1 = op_ps.tile([128, 512], F32, tag="acc")
                    for k in range(NM):
                        nc.tensor.matmul(
                            ps0[:],
                            onorm[:, k * S + i * 128 : k * S + (i + 1) * 128],
                            woT[:, k * E : k * E + 512],
                            start=(k == 0),
                            stop=(k == NM - 1),
                        )
                        nc.tensor.matmul(
                            ps1[:],
                            onorm[:, k * S + i * 128 : k * S + (i + 1) * 128],
                            woT[:, k * E + 512 : k * E + 1024],
                            start=(k == 0),
                            stop=(k == NM - 1),
                        )
                    osb = outp.tile([128, E], F32, tag="osb")
                    nc.vector.tensor_copy(osb[:, 0:512], ps0[:])
                    nc.vector.tensor_copy(osb[:, 512:1024], ps1[:])
                    nc.sync.dma_start(out_d[i * 128 : (i + 1) * 128, :], osb[:])

            attn_stack.close()

    return nc


def _make_sel():
    # zbuf row for head (2k + p//64): even heads -> row k, odd heads -> row 4+k
    sel = np.zeros((HG, 512), dtype=np.float32)
    for k in range(4):
        for p in range(128):
            r = k if p < 64 else 4 + k
            sel[r, k * 128 + p] = 1.0
    return sel


def kernel(x, Wq, Wk, Wv, Wo):
    global LAST_EXEC_TIME_NS
    _env_setup()
    from concourse.bass_utils import run_bass_kernel_spmd

    x = np.asarray(x, dtype=np.float32)
    Wq = np.asarray(Wq, dtype=np.float32)
    Wk = np.asarray(Wk, dtype=np.float32)
    Wv = np.asarray(Wv, dtype=np.float32)
    Wo = np.asarray(Wo, dtype=np.float32)

    if "nc" not in _cache:
        _cache["nc"] = _build_nc()
    nc = _cache["nc"]

    if MM_DTYPE == "bf16":
        import ml_dtypes

        cdt = ml_dtypes.bfloat16
    else:
        cdt = np.float32

    sel = _make_sel()
    in_maps = []
    for c in range(NCORES):
        b, hg = c // 2, c % 2
        r = slice(hg * DG, (hg + 1) * DG)
        in_maps.append(
            {
                "xT": np.ascontiguousarray(x[b].T).astype(cdt),
                "wqT": np.ascontiguousarray(Wq[r, :].T).astype(cdt),
                "wkT": np.ascontiguousarray(Wk[r, :].T).astype(cdt),
                "wvT": np.ascontiguousarray(Wv[r, :].T).astype(cdt),
                "woT": np.ascontiguousarray(Wo[:, r].T).astype(cdt),
                "sel": sel,
            }
        )

    res = run_bass_kernel_spmd(
        nc, in_maps, core_ids=list(range(NCORES)), trace=TRACE
    )
    if TRACE:
        LAST_EXEC_TIME_NS = res.exec_time_ns

    out = np.empty((B, S, E), dtype=np.float32)
    for b in range(B):
        out[b] = res.results[2 * b]["out"] + res.results[2 * b + 1]["out"]
    return out
